# revision 4
# baseline (speedup 1.0000x reference)
"""Trainium2 Bass kernel: quarter-tile pipeline, dual-e4m3 fp8 weights.

Trainium2 Bass kernel for nn_Decoder_60627758350737 (GNN message passing).

Sharding: node dim N=2048 split across 8 cores (256 nodes each); weights
replicated; no collectives (the gather source is g_l = node_features @ W0n_l,
computed from the initial node features on every core).

v3 structure:
- Two node groups of 128 per core, software-pipelined: group g's layer tail
  (LN/dense/LN) is scheduled under the other group's k-loop, so the serial
  tail chain hides behind tensor-engine work.
- agg = (sum_k h1) @ W2: the per-edge W2 matmul is factored out of the
  k-loop; h1 is k-summed into a 1-bank PSUM accumulator via identity matmuls.
- gather tables g_l live in SBUF (bf16, DVE-written); per-chunk gathers are
  SBUF-source dma_gather (no DRAM staging, no HBM traffic).
- all weights + edges bf16; residuals/LN in f32r. b0 folded into the xw
  precompute so the h0 gelu is one batched un-biased activation.
- ScalarE runs ONLY Gelu (single act-table load): Identity-style bias adds
  moved to DVE tensor_scalar; LN rsqrt = DVE Newton iteration (bit-trick
  seed), no Sqrt table.
"""
import numpy as np
import ml_dtypes
import concourse.bass as bass
import concourse.bacc as bacc
import concourse.mybir as mybir
from concourse import tile
from concourse.bass_utils import run_bass_kernel_spmd
from contextlib import ExitStack

F32 = mybir.dt.float32
F32R = mybir.dt.float32r
BF16 = mybir.dt.bfloat16
I16 = mybir.dt.int16
I32 = mybir.dt.int32
F8 = mybir.dt.float8e4
AF = mybir.ActivationFunctionType
OP = mybir.AluOpType
BF = ml_dtypes.bfloat16

N, K, NF, L = 2048, 48, 384, 3
NCORES = 8
NLOC = N // NCORES            # 256
NG = 128                      # nodes per group
TG = NG * K                   # 6144 tokens per group (k-major: t = k*128 + n)
CH = 1024                     # chunk = 8 k-values x 128 nodes
NCH = TG // CH                # 6 chunks per group
SCALE = 30.0
EPS = 1e-5
MAGIC = 0x5F3759DF

# wm blob column offsets (bf16 weights, per layer [128, WMC])
O_W0X = 0
O_W2 = 1152
O_DW0 = 2304
O_DW1 = 6912
WMC = 11520
# wm8 blob (fp8e4 DoubleRow pairs, per layer [128, W8C])
# per mt: [W0e_kt0|W0e_kt1|0|W0e_kt2]*32 then same for W1*16
O8_W0E = 0
O8_W1 = 2304
W8C = 4608
SC_W0E = 32.0
SC_W1 = 16.0
# wb blob (f32 biases, per layer [128, 24])
O_B0 = 0
O_B1 = 3
O_B2 = 6
O_DB0 = 9
O_DB1 = 21

_NC_CACHE = {}


def _emit(act=None, layers=L):
    act = AF.Gelu if act is None else act
    nc = bacc.Bacc(num_swdge_queues=4)
    edge_p = nc.declare_dram_parameter("edge", [2, NCH, 128, 3, CH], F8,
                                       isOutput=False)
    nfm_p = nc.declare_dram_parameter("nfm", [128, 3, N], BF16, isOutput=False)
    gidx_p = nc.declare_dram_parameter("gidx", [128, 2, NCH, CH // 16], I16,
                                       isOutput=False)
    wm_p = nc.declare_dram_parameter("wm", [L, 128, WMC], BF16, isOutput=False)
    wm8_p = nc.declare_dram_parameter("wm8", [L, 128, W8C], F8, isOutput=False)
    wb_p = nc.declare_dram_parameter("wb", [L, 128, 24], F32, isOutput=False)
    wn_p = nc.declare_dram_parameter("wn", [128, L, 3, 384], BF16, isOutput=False)
    ln_p = nc.declare_dram_parameter("lnpk", [L, 1, 1920], F32R, isOutput=False)
    b1r_p = nc.declare_dram_parameter("b1r", [L, 1, 384], F32R, isOutput=False)
    cst_p = nc.declare_dram_parameter("consts", [128, 769], F32R, isOutput=False)
    cstb_p = nc.declare_dram_parameter("constsb", [128, 128], BF16, isOutput=False)
    cst8_p = nc.declare_dram_parameter("consts8", [128, 2, 128], F8, isOutput=False)
    x0_p = nc.declare_dram_parameter("x0", [128, 3, NLOC], F32R, isOutput=False)
    mask_p = nc.declare_dram_parameter("mask", [1, NLOC], F32, isOutput=False)
    out_p = nc.declare_dram_parameter("out_x", [128, 3, NLOC], F32, isOutput=True)

    with tile.TileContext(nc) as tc, ExitStack() as ctx:
        wpool = ctx.enter_context(tc.tile_pool(name="w", bufs=2))
        gpool = ctx.enter_context(tc.tile_pool(name="g", bufs=2))
        spool = ctx.enter_context(tc.tile_pool(name="stream", bufs=2))
        work1 = ctx.enter_context(tc.tile_pool(name="work1", bufs=1))
        work2 = ctx.enter_context(tc.tile_pool(name="work2", bufs=2))
        xpool = ctx.enter_context(tc.tile_pool(name="xp", bufs=2))
        small = ctx.enter_context(tc.tile_pool(name="small", bufs=1))
        dram = ctx.enter_context(tc.tile_pool(name="dram", bufs=1, space="DRAM"))
        dram = ctx.enter_context(tc.tile_pool(name="dram", bufs=1, space="DRAM"))
        mm = ctx.enter_context(tc.tile_pool(name="mm", bufs=3, space="PSUM"))
        aggp = ctx.enter_context(tc.tile_pool(name="aggp", bufs=1, space="PSUM"))
        tpp = ctx.enter_context(tc.tile_pool(name="tpp", bufs=1, space="PSUM"))

        # --- one-time loads ---
        cst = small.tile([128, 769], F32R, tag="cst")
        nc.sync.dma_start(cst[:], cst_p[:])
        cstb = small.tile([128, 128], BF16, tag="cstb")
        nc.sync.dma_start(cstb[:], cstb_p[:])
        cst8 = small.tile([128, 2, 128], F8, tag="cst8")
        nc.sync.dma_start(cst8[:], cst8_p[:])
        nfm = small.tile([128, 3, N], BF16, tag="nfm")
        nc.sync.dma_start(nfm[:], nfm_p[:])
        gidx = small.tile([128, 2, NCH, CH // 16], I16, tag="gidx")
        nc.sync.dma_start(gidx[:], gidx_p[:])
        maskt = small.tile([1, NLOC], F32, tag="maskt")
        nc.sync.dma_start(maskt[:], mask_p[:])
        wn = small.tile([128, L, 3, 384], BF16, tag="wn")
        nc.scalar.dma_start(wn[:], wn_p[:])

        ones_col = cst[:, 0:1]            # [128,1] ones (stats lhsT)
        ones_row = cst[0:1, 1:129]        # [1,128] ones
        ones512 = cst[0:1, 257:769]       # [1,512] ones
        eye_b = cstb[:]                   # [128,128] identity bf16
        eye2 = cst8[:]                    # [128,2,128] identity pair fp8e4

        xg = {}      # (g) -> current residual tile [128,3,128] f32r
        xbg = {}     # bf16 copy for matmul rhs
        for g in range(2):
            xt = xpool.tile([128, 3, NG], F32R, tag=f"x{g}", name=f"x{g}")
            nc.sync.dma_start(xt[:], x0_p[:, :, g * NG:(g + 1) * NG])
            xg[g] = xt
            xbt = xpool.tile([128, 3, NG], BF16, tag=f"xb{g}", name=f"xb{g}")
            nc.vector.tensor_copy(xbt[:], xt[:].bitcast(F32))
            xbg[g] = xbt

        def compute_g(l):
            """g_l = nf0 @ W0n_l, token-major, staged to DRAM [2048,384]."""
            g_s = gpool.tile([128, 16, 384], BF16, tag="gtab", bufs=2,
                             name=f"gtab{l}")
            for tt in range(16):
                gp = mm.tile([128, 3, 256], F32, tag="mm", name="gp")
                for kt in range(3):
                    nc.tensor.matmul(
                        gp[:].rearrange("p a b -> p (a b)")[:, 0:384],
                        nfm[:, kt, tt * 128:(tt + 1) * 128],
                        wn[:, l, kt, :],
                        start=(kt == 0), stop=(kt == 2))
                nc.vector.tensor_copy(
                    g_s[:, tt, :],
                    gp[:].rearrange("p a b -> p (a b)")[:, 0:384])
            g_l = dram.tile([N, 384], BF16, tag=f"gd{l}", name=f"gd{l}")
            nc.sync.dma_start(
                g_l[:].rearrange("(a p) e -> p a e", p=128), g_s[:])
            return g_l

        def load_weights(l):
            wm = wpool.tile([128, WMC], BF16, tag="wm", name=f"wm{l}")
            nc.scalar.dma_start(wm[:], wm_p[l])
            wm8 = wpool.tile([128, W8C], F8, tag="wm8", name=f"wm8{l}")
            nc.scalar.dma_start(wm8[:], wm8_p[l])
            wb = wpool.tile([128, 24], F32, tag="wb", name=f"wb{l}")
            nc.scalar.dma_start(wb[:], wb_p[l])
            lnw = wpool.tile([1, 1920], F32R, tag="lnw", bufs=1,
                             name=f"lnw{l}")
            nc.scalar.dma_start(lnw[:], ln_p[l])
            b1r = wpool.tile([1, 384], F32R, tag="b1r", name=f"b1r{l}")
            nc.scalar.dma_start(b1r[:], b1r_p[l])
            return wm, wm8, wb, lnw, b1r

        def prep_xw(l, g, wm, wb):
            """xw8 = dup8(x_g @ W0x_l + b0) bf16 [128,3,8,128]."""
            xwp = tpp.tile([128, 384], F32, tag="tp", name="xwp")
            for mt in range(3):
                for kt in range(3):
                    nc.tensor.matmul(
                        xwp[:, mt * NG:(mt + 1) * NG],
                        wm[:, O_W0X + kt * 384 + mt * 128: O_W0X + kt * 384 + (mt + 1) * 128],
                        xbg[g][:, kt, :],
                        start=(kt == 0), stop=(kt == 2))
            xw8 = work1.tile([128, 3, 8, NG], BF16, tag=f"xw8{g}",
                             name=f"xw8{g}")
            for mt in range(3):
                nc.vector.tensor_scalar(
                    xw8[:, mt, 0, :], xwp[:, mt * NG:(mt + 1) * NG],
                    wb[:, O_B0 + mt:O_B0 + mt + 1], None, op0=OP.add)
            nc.vector.tensor_copy(xw8[:, :, 1, :], xw8[:, :, 0, :])
            nc.vector.tensor_copy(xw8[:, :, 2:4, :], xw8[:, :, 0:2, :])
            nc.vector.tensor_copy(xw8[:, :, 4:8, :], xw8[:, :, 0:4, :])
            return xw8

        def kloop(l, g, g_l, xw8, wm8, wb, b1r):
            """Accumulate hsum = sum_k h1 for group g. Returns PSUM tile."""
            hsum = aggp.tile([128, 384], F32, tag="agg", name=f"hs{l}{g}")
            for cc in range(NCH):
                gts = []
                for h in range(2):
                    gth = spool.tile([128, 3, 512], BF16, tag=f"gt{h}",
                                     bufs=3, name=f"gt{h}")
                    nc.gpsimd.dma_gather(
                        gth[:], g_l[:],
                        gidx[:, g, cc, h * 32:(h + 1) * 32],
                        num_idxs=512, num_idxs_reg=512, elem_size=384,
                        transpose=True,
                        queue_num=(cc * 2 + h) % 4)
                    nc.vector.tensor_add(
                        gth[:], gth[:],
                        xw8[:, :, 4 * h:4 * h + 4, :]
                        .rearrange("p a b c -> p a (b c)"))
                    gts.append(gth)
                et = spool.tile([128, 3, CH], F8, tag="et", bufs=3, name="et")
                nc.sync.dma_start(et[:], edge_p[g, cc])
                for qq in range(4):
                    hs = slice(qq * 256, (qq + 1) * 256)
                    gt = gts[qq // 2]
                    gs = slice((qq % 2) * 256, (qq % 2) * 256 + 256)
                    h0g = work2.tile([128, 3, 256], F8, tag="h0g", name="h0g")
                    hp = mm.tile([128, 3, 256], F32, tag="mm", name="hp")
                    for mt in range(3):
                        for p8, sl in enumerate(
                                (slice(0, 2), slice(1, 3), slice(0, 3, 2))):
                            nc.tensor.matmul(
                                hp[:, mt, :],
                                wm8[:, O8_W0E + mt * 768 + p8 * 256:
                                    O8_W0E + mt * 768 + (p8 + 1) * 256]
                                .rearrange("p (a b) -> p a b", a=2),
                                et[:, sl, hs],
                                start=(p8 == 0), stop=False,
                                perf_mode=mybir.MatmulPerfMode.DoubleRow)
                        nc.tensor.matmul(hp[:, mt, :], eye_b, gt[:, mt, gs],
                                         start=False, stop=True)
                    nc.scalar.activation(h0g[:], hp[:], act,
                                         scale=1.0 / SC_W0E)
                    h1g = work2.tile([128, 3, 256], F8, tag="h1g", name="h1g")
                    h1p = mm.tile([128, 3, 256], F32, tag="mm", name="h1p")
                    for mt in range(3):
                        nc.tensor.matmul(
                            h1p[:, mt, :],
                            b1r[0:1, mt * 128:(mt + 1) * 128],
                            ones512[0:1, 0:256], start=True, stop=False)
                        for p8, sl in enumerate(
                                (slice(0, 2), slice(1, 3), slice(0, 3, 2))):
                            nc.tensor.matmul(
                                h1p[:, mt, :],
                                wm8[:, O8_W1 + mt * 768 + p8 * 256:
                                    O8_W1 + mt * 768 + (p8 + 1) * 256]
                                .rearrange("p (a b) -> p a b", a=2),
                                h0g[:, sl, :],
                                start=False, stop=(p8 == 2),
                                perf_mode=mybir.MatmulPerfMode.DoubleRow)
                    nc.scalar.activation(h1g[:], h1p[:], act,
                                         scale=1.0 / SC_W1)
                    first = (cc == 0 and qq == 0)
                    last = (cc == NCH - 1 and qq == 3)
                    for mt in range(3):
                        nc.tensor.matmul(
                            hsum[:, mt * NG:(mt + 1) * NG],
                            eye2,
                            h1g[:, mt, :]
                            .rearrange("p (a b) -> p a b", a=2),
                            start=(first and mt == 0),
                            stop=last,
                            perf_mode=mybir.MatmulPerfMode.DoubleRow,
                            skip_group_check=True)
            return hsum

        def rsqrt_row(v):
            """[1,n] f32 SBUF -> [1,n] f32 rstd, DVE-only Newton iteration."""
            n = v.shape[-1]
            yi = small.tile([1, n], F32, tag="yi", name="yi")
            tn = small.tile([1, n], F32, tag="tn", name="tn")
            # seed: bitpattern C - (v_bits >> 1), subtract done in f32 domain
            nc.vector.tensor_scalar(
                yi[:].bitcast(I32), v[:].bitcast(I32), 1, None,
                op0=OP.logical_shift_right)
            nc.vector.tensor_copy(tn[:], yi[:].bitcast(I32))  # int -> f32 value
            nc.vector.tensor_scalar(tn[:], tn[:], -1.0, float(MAGIC),
                                    op0=OP.mult, op1=OP.add)
            nc.vector.tensor_copy(yi[:].bitcast(I32), tn[:])  # f32 -> int bits
            y = yi[:].bitcast(F32)
            for _ in range(3):
                nc.vector.tensor_mul(tn[:], y, y)
                nc.vector.tensor_mul(tn[:], tn[:], v[:])
                nc.vector.tensor_scalar(tn[:], tn[:], -0.5, 1.5,
                                        op0=OP.mult, op1=OP.add)
                nc.vector.tensor_mul(y, y, tn[:])
            return yi

        def layernorm(src, lnw, ln_i, g, masked, tag):
            """src: [128,3,128] F32R tile -> new [128,3,128] f32r tile."""
            maskg = maskt[0:1, g * NG:(g + 1) * NG]
            sq = work1.tile([128, 3, NG], F32R, tag="sq", name="sq")
            nc.vector.tensor_mul(sq[:], src[:].bitcast(F32), src[:].bitcast(F32))
            st = tpp.tile([128, 384], F32, tag="tp", name="st")
            for kt in range(3):
                nc.tensor.matmul(st[0:1, 0:NG], ones_col, src[:, kt, :],
                                 start=(kt == 0), stop=(kt == 2))
            for kt in range(3):
                nc.tensor.matmul(st[0:1, NG:2 * NG], ones_col, sq[:, kt, :],
                                 start=(kt == 0), stop=(kt == 2))
            sm = small.tile([1, 2 * NG], F32, tag="sm", name="sm")
            nc.vector.tensor_scalar_mul(sm[:], st[0:1, 0:2 * NG], 1.0 / NF)
            var = small.tile([1, NG], F32, tag="var", name="var")
            nc.vector.tensor_mul(var[:], sm[0:1, 0:NG], sm[0:1, 0:NG])
            nc.vector.tensor_sub(var[:], sm[0:1, NG:2 * NG], var[:])
            nc.vector.tensor_scalar_add(var[:], var[:], EPS)
            rstd = rsqrt_row(var)
            # rv rows (f32r): [0:128]=rstd(*m), [128:256]=-mean*rstd(*m),
            # [256:384]=ones or mask
            rv = small.tile([1, 384], F32R, tag="rv", name="rv")
            nmr = small.tile([1, NG], F32, tag="nmr", name="nmr")
            nc.vector.tensor_scalar(nmr[:], sm[0:1, 0:NG], -1.0, None,
                                    op0=OP.mult)
            nc.vector.tensor_mul(nmr[:], nmr[:], rstd[:].bitcast(F32))
            if masked:
                nc.vector.tensor_mul(rv[0:1, 0:NG], rstd[:].bitcast(F32), maskg)
                nc.vector.tensor_mul(rv[0:1, NG:2 * NG], nmr[:], maskg)
                nc.vector.tensor_copy(rv[0:1, 2 * NG:3 * NG], maskg)
            else:
                nc.vector.tensor_copy(rv[0:1, 0:NG], rstd[:].bitcast(F32))
                nc.vector.tensor_copy(rv[0:1, NG:2 * NG], nmr[:])
                nc.vector.tensor_copy(rv[0:1, 2 * NG:3 * NG],
                                      ones_row.bitcast(F32))
            outt = xpool.tile([128, 3, NG], F32R, tag=tag, name=tag)
            stp = tpp.tile([128, 384], F32, tag="tp", name="stS")
            for mt in range(3):
                woff = ln_i * 384 + mt * 128
                nc.tensor.matmul(stp[:, mt * NG:(mt + 1) * NG],
                                 lnw[0:1, woff:woff + 128],
                                 rv[0:1, 0:NG], start=True, stop=True)
            nc.vector.tensor_mul(
                outt[:], src[:].bitcast(F32),
                stp[:].rearrange("p (a b) -> p a b", a=3))
            stp2 = tpp.tile([128, 384], F32, tag="tp", name="stT")
            for mt in range(3):
                woff = ln_i * 384 + mt * 128
                nc.tensor.matmul(stp2[:, mt * NG:(mt + 1) * NG],
                                 lnw[0:1, 768 + woff:768 + woff + 128],
                                 rv[0:1, 2 * NG:3 * NG], start=True, stop=False)
                nc.tensor.matmul(stp2[:, mt * NG:(mt + 1) * NG],
                                 lnw[0:1, woff:woff + 128],
                                 rv[0:1, NG:2 * NG], start=False, stop=True)
            nc.vector.tensor_add(
                outt[:], outt[:].bitcast(F32),
                stp2[:].rearrange("p (a b) -> p a b", a=3))
            return outt

        def tail(l, g, hsum, wm, wb, lnw, wm_next, wb_next):
            """agg -> LN1 -> dense -> LN2 -> x(l+1,g); xw8 prep for l+1."""
            hsum_s = work1.tile([128, 3, NG], BF16, tag="hsum_s", name="hsum_s")
            nc.vector.tensor_copy(
                hsum_s[:], hsum[:].rearrange("p (a b) -> p a b", a=3))
            aggm = tpp.tile([128, 384], F32, tag="tp", name="aggm")
            for mt in range(3):
                for kt in range(3):
                    nc.tensor.matmul(
                        aggm[:, mt * NG:(mt + 1) * NG],
                        wm[:, O_W2 + kt * 384 + mt * 128: O_W2 + kt * 384 + (mt + 1) * 128],
                        hsum_s[:, kt, :],
                        start=(kt == 0), stop=(kt == 2))
            # x1p = x + aggm/SCALE + b2*K/SCALE
            x1p = work1.tile([128, 3, NG], F32R, tag="x1p", name="x1p")
            for mt in range(3):
                nc.vector.tensor_scalar(
                    x1p[:, mt, :], aggm[:, mt * NG:(mt + 1) * NG],
                    1.0 / SCALE, b2s[:, mt:mt + 1], op0=OP.mult, op1=OP.add)
            nc.vector.tensor_add(x1p[:], x1p[:].bitcast(F32),
                                 xg[g][:].bitcast(F32))
            x1 = layernorm(x1p, lnw, 0, g, masked=False, tag=f"x1_{g}")
            x1b = work1.tile([128, 3, NG], BF16, tag=f"x1b{g}", bufs=2,
                             name="x1b")
            nc.vector.tensor_copy(x1b[:], x1[:].bitcast(F32))

            # dense MLP: d0 = gelu(x1 @ dw0 + db0); d1 = d0 @ dw1 + db1
            d0g = work1.tile([128, 12, NG], BF16, tag="d0g", name="d0g")
            for r in range(4):
                dp = tpp.tile([128, 384], F32, tag="tp", name="dp")
                for j in range(3):
                    mt = r * 3 + j
                    reg = dp[:, j * NG:(j + 1) * NG]
                    for kt in range(3):
                        nc.tensor.matmul(
                            reg,
                            wm[:, O_DW0 + kt * 1536 + mt * 128: O_DW0 + kt * 1536 + (mt + 1) * 128],
                            x1b[:, kt, :],
                            start=(kt == 0), stop=(kt == 2))
                    nc.scalar.activation(d0g[:, mt, :], reg, act,
                                         bias=wb[:, O_DB0 + mt:O_DB0 + mt + 1])
            d1p = tpp.tile([128, 384], F32, tag="tp", name="d1p")
            for mt in range(3):
                for kt in range(12):
                    nc.tensor.matmul(
                        d1p[:, mt * NG:(mt + 1) * NG],
                        wm[:, O_DW1 + kt * 384 + mt * 128: O_DW1 + kt * 384 + (mt + 1) * 128],
                        d0g[:, kt, :],
                        start=(kt == 0), stop=(kt == 11))
            x2p = work1.tile([128, 3, NG], F32R, tag="x2p", name="x2p")
            for mt in range(3):
                nc.vector.tensor_scalar(
                    x2p[:, mt, :], d1p[:, mt * NG:(mt + 1) * NG],
                    1.0, wb[:, O_DB1 + mt:O_DB1 + mt + 1],
                    op0=OP.mult, op1=OP.add)
            nc.vector.tensor_add(x2p[:], x2p[:].bitcast(F32),
                                 x1[:].bitcast(F32))
            xo = layernorm(x2p, lnw, 1, g, masked=True, tag=f"x{g}")
            xg[g] = xo
            if l + 1 < layers:
                xb = xpool.tile([128, 3, NG], BF16, tag=f"xb{g}", name=f"xb{g}")
                nc.vector.tensor_copy(xb[:], xo[:].bitcast(F32))
                xbg[g] = xb
                return prep_xw(l + 1, g, wm_next, wb_next)
            nc.sync.dma_start(out_p[:, :, g * NG:(g + 1) * NG],
                              xo[:].bitcast(F32))
            return None

        # ================= pipeline =================
        g_tiles = {0: compute_g(0), 1: compute_g(1), 2: compute_g(2)}
        wms = {0: load_weights(0), 1: load_weights(1)}
        b2s_all = {}

        def get_b2s(l, wb):
            if l not in b2s_all:
                t = small.tile([128, 3], F32, tag=f"b2s{l % 2}", name=f"b2s{l}")
                nc.vector.tensor_scalar_mul(t[:], wb[:, O_B2:O_B2 + 3],
                                            K / SCALE)
                b2s_all[l] = t
            return b2s_all[l]

        xw8s = {}
        wm0, _, wb0, _, _ = wms[0]
        for g in range(2):
            xw8s[g] = prep_xw(0, g, wm0, wb0)

        for l in range(layers):
            wm, wm8, wb, lnw, b1r = wms[l]
            b2s = get_b2s(l, wb)
            if l + 1 < layers:
                if l + 1 not in wms:
                    wms[l + 1] = load_weights(l + 1)
                wm_next, _, wb_next, _, _ = wms[l + 1]
            else:
                wm_next = wb_next = None
            for g in range(2):
                hsum = kloop(l, g, g_tiles[l], xw8s[g], wm8, wb, b1r)
                xw8s[g] = tail(l, g, hsum, wm, wb, lnw, wm_next, wb_next)

    nc.finalize()
    return nc


def _get_nc():
    if "nc" not in _NC_CACHE:
        _NC_CACHE["nc"] = _emit()
    return _NC_CACHE["nc"]


def _fm(w):
    """[in, out] fp32 -> [128, n_kt*out] (feature-major lhsT blob columns)."""
    i, o = w.shape
    return np.ascontiguousarray(
        w.reshape(i // 128, 128, o).transpose(1, 0, 2).reshape(128, -1))


def _wrap_idx(vals):
    """[n] int -> [128, n//16] int16 wrapped (i -> [i%16, i//16]) x8 replicas."""
    n = vals.shape[0]
    w = np.ascontiguousarray(vals.reshape(n // 16, 16).T).astype(np.int16)
    return np.tile(w, (8, 1))


def _marshal(inputs):
    nf = np.asarray(inputs["node_features"], np.float32)
    ef = np.asarray(inputs["edge_features"], np.float32)
    idx = np.asarray(inputs["neighbor_indices"])
    mask = np.asarray(inputs["mask"], np.float32)

    # replicated tensors
    nfm = np.ascontiguousarray(
        nf.astype(BF).reshape(N, 3, 128).transpose(2, 1, 0))           # [128,3,N]
    f8np = mybir.dt.np(mybir.dt.float8e4)
    wm = np.empty((L, 128, WMC), BF)
    wm8 = np.empty((L, 128, W8C), f8np)
    wb = np.empty((L, 128, 24), np.float32)
    wn = np.empty((128, L, 3, 384), BF)
    lnpk = np.empty((L, 1, 1920), np.float32)
    b1r_m = np.empty((L, 1, 384), np.float32)
    for l in range(L):
        w0 = np.asarray(inputs["msg_w0"], np.float32)[l]
        cols = [
            _fm(w0[0:384]),
            _fm(np.asarray(inputs["msg_w2"], np.float32)[l]),
            _fm(np.asarray(inputs["dense_w0"], np.float32)[l]),
            _fm(np.asarray(inputs["dense_w1"], np.float32)[l]),
        ]
        wm[l] = np.concatenate(cols, axis=1).astype(BF)
        w0e = _fm(w0[384:768])
        w1f = _fm(np.asarray(inputs["msg_w1"], np.float32)[l])
        c8 = []
        for W, sc in ((w0e, SC_W0E), (w1f, SC_W1)):
            q = (W * sc).astype(f8np)
            d = (W * sc - q.astype(np.float32)).astype(f8np)
            for mt in range(3):
                blk = lambda A, kt: A[:, kt * 384 + mt * 128:
                                      kt * 384 + (mt + 1) * 128]
                c8 += [blk(q, 0), blk(q, 1), blk(d, 1), blk(d, 2),
                       blk(d, 0), blk(q, 2)]
        wm8[l] = np.concatenate(
            [c.astype(f8np) for c in c8], axis=1)
        bcols = [
            np.asarray(inputs["msg_b0"], np.float32)[l].reshape(3, 128).T,
            np.asarray(inputs["msg_b1"], np.float32)[l].reshape(3, 128).T,
            np.asarray(inputs["msg_b2"], np.float32)[l].reshape(3, 128).T,
            np.asarray(inputs["dense_b0"], np.float32)[l].reshape(12, 128).T,
            np.asarray(inputs["dense_b1"], np.float32)[l].reshape(3, 128).T,
        ]
        wb[l] = np.concatenate(bcols, axis=1)
        wn[:, l] = w0[1152:1536].astype(BF).reshape(3, 128, 384).transpose(1, 0, 2)
        lnpk[l, 0] = np.concatenate([
            np.asarray(inputs["ln1_w"], np.float32)[l],
            np.asarray(inputs["ln2_w"], np.float32)[l],
            np.asarray(inputs["ln1_b"], np.float32)[l],
            np.asarray(inputs["ln2_b"], np.float32)[l],
            np.asarray(inputs["msg_b1"], np.float32)[l] * SC_W1])
        b1r_m[l, 0] = np.asarray(inputs["msg_b1"], np.float32)[l] * SC_W1
    consts = np.zeros((128, 769), np.float32)
    consts[:, 0] = 1.0
    consts[0, 1:769] = 1.0
    constsb = (np.eye(128, dtype=np.float32) * SC_W0E).astype(BF)
    consts8 = np.broadcast_to(np.eye(128, dtype=np.float32), (2, 128, 128))
    consts8 = np.ascontiguousarray(
        consts8.transpose(1, 0, 2)).astype(f8np)

    in_maps = []
    for c in range(NCORES):
        lo = slice(c * NLOC, (c + 1) * NLOC)
        efc = ef[lo]                                       # [256,48,384]
        idc = idx[lo]                                      # [256,48]
        edge = np.empty((2, NCH, 128, 3, CH), f8np)
        gidx = np.empty((128, 2, NCH, CH // 16), np.int16)
        for g in range(2):
            gs = slice(g * NG, (g + 1) * NG)
            E = efc[gs].transpose(1, 0, 2).reshape(TG, 384)    # k-major tokens
            edge[g] = (E.reshape(NCH, 8, NG, 3, 128)
                       .transpose(0, 4, 3, 1, 2).reshape(NCH, 128, 3, CH)
                       .astype(f8np))
            idx_k = np.ascontiguousarray(idc[gs].T).reshape(TG)
            for cc in range(NCH):
                gidx[:, g, cc, :] = _wrap_idx(idx_k[cc * CH:(cc + 1) * CH])
        x0 = np.ascontiguousarray(
            nf[lo].reshape(NLOC, 3, 128).transpose(2, 1, 0))   # [128,3,256]
        in_maps.append(dict(
            edge=edge, nfm=nfm, gidx=gidx, wm=wm, wb=wb, wn=wn, lnpk=lnpk,
            consts=consts, constsb=constsb, consts8=consts8, wm8=wm8,
            b1r=b1r_m, x0=x0,
            mask=np.ascontiguousarray(mask[lo])[None, :]))
    return in_maps


def _unshard(results):
    out = np.empty((N, NF), np.float32)
    for c in range(NCORES):
        xfm = results[c]["out_x"]                          # [128,3,256]
        out[c * NLOC:(c + 1) * NLOC] = xfm.transpose(2, 1, 0).reshape(NLOC, NF)
    return out


def kernel(**inputs):
    nc = _get_nc()
    in_maps = _marshal(inputs)
    res = run_bass_kernel_spmd(nc, in_maps, list(range(NCORES)), trace=False)
    return _unshard(res.results)


# revision 5
# speedup vs baseline: 1.0036x; 1.0036x over previous
"""Trainium2 Bass kernel: quarter-tile pipeline, dual-e4m3 fp8 weights.

Trainium2 Bass kernel for nn_Decoder_60627758350737 (GNN message passing).

Sharding: node dim N=2048 split across 8 cores (256 nodes each); weights
replicated; no collectives (the gather source is g_l = node_features @ W0n_l,
computed from the initial node features on every core).

v3 structure:
- Two node groups of 128 per core, software-pipelined: group g's layer tail
  (LN/dense/LN) is scheduled under the other group's k-loop, so the serial
  tail chain hides behind tensor-engine work.
- agg = (sum_k h1) @ W2: the per-edge W2 matmul is factored out of the
  k-loop; h1 is k-summed into a 1-bank PSUM accumulator via identity matmuls.
- gather tables g_l live in SBUF (bf16, DVE-written); per-chunk gathers are
  SBUF-source dma_gather (no DRAM staging, no HBM traffic).
- all weights + edges bf16; residuals/LN in f32r. b0 folded into the xw
  precompute so the h0 gelu is one batched un-biased activation.
- ScalarE runs ONLY Gelu (single act-table load): Identity-style bias adds
  moved to DVE tensor_scalar; LN rsqrt = DVE Newton iteration (bit-trick
  seed), no Sqrt table.
"""
import numpy as np
import ml_dtypes
import concourse.bass as bass
import concourse.bacc as bacc
import concourse.mybir as mybir
from concourse import tile
from concourse.bass_utils import run_bass_kernel_spmd
from contextlib import ExitStack

F32 = mybir.dt.float32
F32R = mybir.dt.float32r
BF16 = mybir.dt.bfloat16
I16 = mybir.dt.int16
I32 = mybir.dt.int32
F8 = mybir.dt.float8e4
AF = mybir.ActivationFunctionType
OP = mybir.AluOpType
BF = ml_dtypes.bfloat16

N, K, NF, L = 2048, 48, 384, 3
NCORES = 8
NLOC = N // NCORES            # 256
NG = 128                      # nodes per group
TG = NG * K                   # 6144 tokens per group (k-major: t = k*128 + n)
CH = 1024                     # chunk = 8 k-values x 128 nodes
NCH = TG // CH                # 6 chunks per group
SCALE = 30.0
EPS = 1e-5
MAGIC = 0x5F3759DF

# wm blob column offsets (bf16 weights, per layer [128, WMC])
O_W0X = 0
O_W2 = 1152
O_DW0 = 2304
O_DW1 = 6912
WMC = 11520
# wm8 blob (fp8e4 DoubleRow pairs, per layer [128, W8C])
# per mt: [W0e_kt0|W0e_kt1|0|W0e_kt2]*32 then same for W1*16
O8_W0E = 0
O8_W1 = 2304
W8C = 4608
SC_W0E = 32.0
SC_W1 = 16.0
# wb blob (f32 biases, per layer [128, 24])
O_B0 = 0
O_B1 = 3
O_B2 = 6
O_DB0 = 9
O_DB1 = 21

_NC_CACHE = {}


def _emit(act=None, layers=L):
    act = AF.Gelu if act is None else act
    nc = bacc.Bacc(num_swdge_queues=4)
    edge_p = nc.declare_dram_parameter("edge", [2, NCH, 128, 3, CH], F8,
                                       isOutput=False)
    nfm_p = nc.declare_dram_parameter("nfm", [128, 3, N], BF16, isOutput=False)
    gidx_p = nc.declare_dram_parameter("gidx", [128, 2, NCH, CH // 16], I16,
                                       isOutput=False)
    wm_p = nc.declare_dram_parameter("wm", [L, 128, WMC], BF16, isOutput=False)
    wm8_p = nc.declare_dram_parameter("wm8", [L, 128, W8C], F8, isOutput=False)
    wb_p = nc.declare_dram_parameter("wb", [L, 128, 24], F32, isOutput=False)
    wn_p = nc.declare_dram_parameter("wn", [128, L, 3, 384], BF16, isOutput=False)
    ln_p = nc.declare_dram_parameter("lnpk", [L, 1, 1920], F32R, isOutput=False)
    b1r_p = nc.declare_dram_parameter("b1r", [L, 1, 384], F32R, isOutput=False)
    cst_p = nc.declare_dram_parameter("consts", [128, 769], F32R, isOutput=False)
    cstb_p = nc.declare_dram_parameter("constsb", [128, 128], BF16, isOutput=False)
    cst8_p = nc.declare_dram_parameter("consts8", [128, 2, 128], F8, isOutput=False)
    x0_p = nc.declare_dram_parameter("x0", [128, 3, NLOC], F32R, isOutput=False)
    mask_p = nc.declare_dram_parameter("mask", [1, NLOC], F32, isOutput=False)
    out_p = nc.declare_dram_parameter("out_x", [128, 3, NLOC], F32, isOutput=True)

    with tile.TileContext(nc) as tc, ExitStack() as ctx:
        wpool = ctx.enter_context(tc.tile_pool(name="w", bufs=2))
        gpool = ctx.enter_context(tc.tile_pool(name="g", bufs=2))
        spool = ctx.enter_context(tc.tile_pool(name="stream", bufs=2))
        work1 = ctx.enter_context(tc.tile_pool(name="work1", bufs=1))
        work2 = ctx.enter_context(tc.tile_pool(name="work2", bufs=2))
        xpool = ctx.enter_context(tc.tile_pool(name="xp", bufs=2))
        small = ctx.enter_context(tc.tile_pool(name="small", bufs=1))
        dram = ctx.enter_context(tc.tile_pool(name="dram", bufs=1, space="DRAM"))
        dram = ctx.enter_context(tc.tile_pool(name="dram", bufs=1, space="DRAM"))
        mm = ctx.enter_context(tc.tile_pool(name="mm", bufs=3, space="PSUM"))
        aggp = ctx.enter_context(tc.tile_pool(name="aggp", bufs=1, space="PSUM"))
        tpp = ctx.enter_context(tc.tile_pool(name="tpp", bufs=1, space="PSUM"))

        # --- one-time loads ---
        cst = small.tile([128, 769], F32R, tag="cst")
        nc.sync.dma_start(cst[:], cst_p[:])
        cstb = small.tile([128, 128], BF16, tag="cstb")
        nc.sync.dma_start(cstb[:], cstb_p[:])
        cst8 = small.tile([128, 2, 128], F8, tag="cst8")
        nc.sync.dma_start(cst8[:], cst8_p[:])
        nfm = small.tile([128, 3, N], BF16, tag="nfm")
        nc.sync.dma_start(nfm[:], nfm_p[:])
        gidx = small.tile([128, 2, NCH, CH // 16], I16, tag="gidx")
        nc.sync.dma_start(gidx[:], gidx_p[:])
        maskt = small.tile([1, NLOC], F32, tag="maskt")
        nc.sync.dma_start(maskt[:], mask_p[:])
        wn = small.tile([128, L, 3, 384], BF16, tag="wn")
        nc.scalar.dma_start(wn[:], wn_p[:])

        ones_col = cst[:, 0:1]            # [128,1] ones (stats lhsT)
        ones_row = cst[0:1, 1:129]        # [1,128] ones
        ones512 = cst[0:1, 257:769]       # [1,512] ones
        eye_b = cstb[:]                   # [128,128] identity bf16
        eye2 = cst8[:]                    # [128,2,128] identity pair fp8e4

        xg = {}      # (g) -> current residual tile [128,3,128] f32r
        xbg = {}     # bf16 copy for matmul rhs
        for g in range(2):
            xt = xpool.tile([128, 3, NG], F32R, tag=f"x{g}", name=f"x{g}")
            nc.sync.dma_start(xt[:], x0_p[:, :, g * NG:(g + 1) * NG])
            xg[g] = xt
            xbt = xpool.tile([128, 3, NG], BF16, tag=f"xb{g}", name=f"xb{g}")
            nc.vector.tensor_copy(xbt[:], xt[:].bitcast(F32))
            xbg[g] = xbt

        def compute_g(l):
            """g_l = nf0 @ W0n_l, token-major, staged to DRAM [2048,384]."""
            g_s = gpool.tile([128, 16, 384], BF16, tag="gtab", bufs=2,
                             name=f"gtab{l}")
            for tt in range(16):
                gp = mm.tile([128, 3, 256], F32, tag="mm", name="gp")
                for kt in range(3):
                    nc.tensor.matmul(
                        gp[:].rearrange("p a b -> p (a b)")[:, 0:384],
                        nfm[:, kt, tt * 128:(tt + 1) * 128],
                        wn[:, l, kt, :],
                        start=(kt == 0), stop=(kt == 2))
                nc.vector.tensor_copy(
                    g_s[:, tt, :],
                    gp[:].rearrange("p a b -> p (a b)")[:, 0:384])
            g_l = dram.tile([N, 384], BF16, tag=f"gd{l}", name=f"gd{l}")
            nc.sync.dma_start(
                g_l[:].rearrange("(a p) e -> p a e", p=128), g_s[:])
            return g_l

        def load_weights(l):
            wm = wpool.tile([128, WMC], BF16, tag="wm", name=f"wm{l}")
            nc.scalar.dma_start(wm[:], wm_p[l])
            wm8 = wpool.tile([128, W8C], F8, tag="wm8", name=f"wm8{l}")
            nc.scalar.dma_start(wm8[:], wm8_p[l])
            wb = wpool.tile([128, 24], F32, tag="wb", name=f"wb{l}")
            nc.scalar.dma_start(wb[:], wb_p[l])
            lnw = wpool.tile([1, 1920], F32R, tag="lnw", bufs=1,
                             name=f"lnw{l}")
            nc.scalar.dma_start(lnw[:], ln_p[l])
            b1r = wpool.tile([1, 384], F32R, tag="b1r", name=f"b1r{l}")
            nc.scalar.dma_start(b1r[:], b1r_p[l])
            return wm, wm8, wb, lnw, b1r

        def prep_xw(l, g, wm, wb):
            """xw8 = dup8(x_g @ W0x_l + b0) bf16 [128,3,8,128]."""
            xwp = tpp.tile([128, 384], F32, tag="tp", name="xwp")
            for mt in range(3):
                for kt in range(3):
                    nc.tensor.matmul(
                        xwp[:, mt * NG:(mt + 1) * NG],
                        wm[:, O_W0X + kt * 384 + mt * 128: O_W0X + kt * 384 + (mt + 1) * 128],
                        xbg[g][:, kt, :],
                        start=(kt == 0), stop=(kt == 2))
            xw8 = work1.tile([128, 3, 8, NG], BF16, tag=f"xw8{g}",
                             name=f"xw8{g}")
            for mt in range(3):
                nc.vector.tensor_scalar(
                    xw8[:, mt, 0, :], xwp[:, mt * NG:(mt + 1) * NG],
                    wb[:, O_B0 + mt:O_B0 + mt + 1], None, op0=OP.add)
            nc.vector.tensor_copy(xw8[:, :, 1, :], xw8[:, :, 0, :])
            nc.vector.tensor_copy(xw8[:, :, 2:4, :], xw8[:, :, 0:2, :])
            nc.vector.tensor_copy(xw8[:, :, 4:8, :], xw8[:, :, 0:4, :])
            return xw8

        def kloop(l, g, g_l, xw8, wm8, wb, b1r):
            """Accumulate hsum = sum_k h1 for group g. Returns PSUM tile."""
            hsum = aggp.tile([128, 384], F32, tag="agg", name=f"hs{l}{g}")
            for cc in range(NCH):
                gts = []
                for h in range(2):
                    gth = spool.tile([128, 3, 512], BF16, tag=f"gt{h}",
                                     bufs=3, name=f"gt{h}")
                    nc.gpsimd.dma_gather(
                        gth[:], g_l[:],
                        gidx[:, g, cc, h * 32:(h + 1) * 32],
                        num_idxs=512, num_idxs_reg=512, elem_size=384,
                        transpose=True,
                        queue_num=(cc * 2 + h) % 4)
                    nc.vector.tensor_add(
                        gth[:], gth[:],
                        xw8[:, :, 4 * h:4 * h + 4, :]
                        .rearrange("p a b c -> p a (b c)"))
                    gts.append(gth)
                et = spool.tile([128, 3, CH], F8, tag="et", bufs=3, name="et")
                nc.sync.dma_start(et[:], edge_p[g, cc])
                for qq in range(4):
                    hs = slice(qq * 256, (qq + 1) * 256)
                    gt = gts[qq // 2]
                    gs = slice((qq % 2) * 256, (qq % 2) * 256 + 256)
                    h0g = work2.tile([128, 3, 256], F8, tag="h0g", name="h0g")
                    hp = mm.tile([128, 3, 256], F32, tag="mm", name="hp")
                    for mt in range(3):
                        for p8, sl in enumerate(
                                (slice(0, 2), slice(1, 3), slice(0, 3, 2))):
                            nc.tensor.matmul(
                                hp[:, mt, :],
                                wm8[:, O8_W0E + mt * 768 + p8 * 256:
                                    O8_W0E + mt * 768 + (p8 + 1) * 256]
                                .rearrange("p (a b) -> p a b", a=2),
                                et[:, sl, hs],
                                start=(p8 == 0), stop=False,
                                perf_mode=mybir.MatmulPerfMode.DoubleRow)
                        nc.tensor.matmul(hp[:, mt, :], eye_b, gt[:, mt, gs],
                                         start=False, stop=True)
                    nc.scalar.activation(h0g[:], hp[:], act,
                                         scale=1.0 / SC_W0E)
                    h1g = work2.tile([128, 3, 256], F8, tag="h1g", name="h1g")
                    h1p = mm.tile([128, 3, 256], F32, tag="mm", name="h1p")
                    for mt in range(3):
                        nc.tensor.matmul(
                            h1p[:, mt, :],
                            b1r[0:1, mt * 128:(mt + 1) * 128],
                            ones512[0:1, 0:256], start=True, stop=False)
                        for p8, sl in enumerate(
                                (slice(0, 2), slice(1, 3), slice(0, 3, 2))):
                            nc.tensor.matmul(
                                h1p[:, mt, :],
                                wm8[:, O8_W1 + mt * 768 + p8 * 256:
                                    O8_W1 + mt * 768 + (p8 + 1) * 256]
                                .rearrange("p (a b) -> p a b", a=2),
                                h0g[:, sl, :],
                                start=False, stop=(p8 == 2),
                                perf_mode=mybir.MatmulPerfMode.DoubleRow)
                    nc.scalar.activation(h1g[:], h1p[:], act,
                                         scale=1.0 / SC_W1)
                    first = (cc == 0 and qq == 0)
                    last = (cc == NCH - 1 and qq == 3)
                    for mt in range(3):
                        nc.tensor.matmul(
                            hsum[:, mt * NG:(mt + 1) * NG],
                            eye2,
                            h1g[:, mt, :]
                            .rearrange("p (a b) -> p a b", a=2),
                            start=(first and mt == 0),
                            stop=last,
                            perf_mode=mybir.MatmulPerfMode.DoubleRow,
                            skip_group_check=True)
            return hsum

        def rsqrt_row(v):
            """[1,n] f32 SBUF -> [1,n] f32 rstd, DVE-only Newton iteration."""
            n = v.shape[-1]
            yi = small.tile([1, n], F32, tag="yi", name="yi")
            tn = small.tile([1, n], F32, tag="tn", name="tn")
            # seed: bitpattern C - (v_bits >> 1), subtract done in f32 domain
            nc.vector.tensor_scalar(
                yi[:].bitcast(I32), v[:].bitcast(I32), 1, None,
                op0=OP.logical_shift_right)
            nc.vector.tensor_copy(tn[:], yi[:].bitcast(I32))  # int -> f32 value
            nc.vector.tensor_scalar(tn[:], tn[:], -1.0, float(MAGIC),
                                    op0=OP.mult, op1=OP.add)
            nc.vector.tensor_copy(yi[:].bitcast(I32), tn[:])  # f32 -> int bits
            y = yi[:].bitcast(F32)
            for _ in range(3):
                nc.vector.tensor_mul(tn[:], y, y)
                nc.vector.tensor_mul(tn[:], tn[:], v[:])
                nc.vector.tensor_scalar(tn[:], tn[:], -0.5, 1.5,
                                        op0=OP.mult, op1=OP.add)
                nc.vector.tensor_mul(y, y, tn[:])
            return yi

        def layernorm(src, lnw, ln_i, g, masked, tag, tp_tile):
            """src: [128,3,128] F32R tile -> new [128,3,128] f32r tile."""
            maskg = maskt[0:1, g * NG:(g + 1) * NG]
            sq = work1.tile([128, 3, NG], F32R, tag="sq", name="sq")
            nc.vector.tensor_mul(sq[:], src[:].bitcast(F32), src[:].bitcast(F32))
            st = tp_tile("st")
            for kt in range(3):
                nc.tensor.matmul(st[0:1, 0:NG], ones_col, src[:, kt, :],
                                 start=(kt == 0), stop=(kt == 2))
            for kt in range(3):
                nc.tensor.matmul(st[0:1, NG:2 * NG], ones_col, sq[:, kt, :],
                                 start=(kt == 0), stop=(kt == 2))
            sm = small.tile([1, 2 * NG], F32, tag="sm", name="sm")
            nc.vector.tensor_scalar_mul(sm[:], st[0:1, 0:2 * NG], 1.0 / NF)
            var = small.tile([1, NG], F32, tag="var", name="var")
            nc.vector.tensor_mul(var[:], sm[0:1, 0:NG], sm[0:1, 0:NG])
            nc.vector.tensor_sub(var[:], sm[0:1, NG:2 * NG], var[:])
            nc.vector.tensor_scalar_add(var[:], var[:], EPS)
            rstd = rsqrt_row(var)
            # rv rows (f32r): [0:128]=rstd(*m), [128:256]=-mean*rstd(*m),
            # [256:384]=ones or mask
            rv = small.tile([1, 384], F32R, tag="rv", name="rv")
            nmr = small.tile([1, NG], F32, tag="nmr", name="nmr")
            nc.vector.tensor_scalar(nmr[:], sm[0:1, 0:NG], -1.0, None,
                                    op0=OP.mult)
            nc.vector.tensor_mul(nmr[:], nmr[:], rstd[:].bitcast(F32))
            if masked:
                nc.vector.tensor_mul(rv[0:1, 0:NG], rstd[:].bitcast(F32), maskg)
                nc.vector.tensor_mul(rv[0:1, NG:2 * NG], nmr[:], maskg)
                nc.vector.tensor_copy(rv[0:1, 2 * NG:3 * NG], maskg)
            else:
                nc.vector.tensor_copy(rv[0:1, 0:NG], rstd[:].bitcast(F32))
                nc.vector.tensor_copy(rv[0:1, NG:2 * NG], nmr[:])
                nc.vector.tensor_copy(rv[0:1, 2 * NG:3 * NG],
                                      ones_row.bitcast(F32))
            outt = xpool.tile([128, 3, NG], F32R, tag=tag, name=tag)
            stp = tp_tile("stS")
            for mt in range(3):
                woff = ln_i * 384 + mt * 128
                nc.tensor.matmul(stp[:, mt * NG:(mt + 1) * NG],
                                 lnw[0:1, woff:woff + 128],
                                 rv[0:1, 0:NG], start=True, stop=True)
            nc.vector.tensor_mul(
                outt[:], src[:].bitcast(F32),
                stp[:].rearrange("p (a b) -> p a b", a=3))
            stp2 = tp_tile("stT")
            for mt in range(3):
                woff = ln_i * 384 + mt * 128
                nc.tensor.matmul(stp2[:, mt * NG:(mt + 1) * NG],
                                 lnw[0:1, 768 + woff:768 + woff + 128],
                                 rv[0:1, 2 * NG:3 * NG], start=True, stop=False)
                nc.tensor.matmul(stp2[:, mt * NG:(mt + 1) * NG],
                                 lnw[0:1, woff:woff + 128],
                                 rv[0:1, NG:2 * NG], start=False, stop=True)
            nc.vector.tensor_add(
                outt[:], outt[:].bitcast(F32),
                stp2[:].rearrange("p (a b) -> p a b", a=3))
            return outt

        def tail(l, g, hsum, wm, wb, lnw, wm_next, wb_next, final=False):

            def tp_tile(name):
                if final:
                    t = mm.tile([128, 3, 256], F32, tag="mm", name=name)
                    return t[:].rearrange("p a b -> p (a b)")[:, 0:384]
                return tpp.tile([128, 384], F32, tag="tp", name=name)
            """agg -> LN1 -> dense -> LN2 -> x(l+1,g); xw8 prep for l+1."""
            hsum_s = work1.tile([128, 3, NG], BF16, tag="hsum_s", name="hsum_s")
            nc.vector.tensor_copy(
                hsum_s[:], hsum[:].rearrange("p (a b) -> p a b", a=3))
            aggm = tp_tile("aggm")
            for mt in range(3):
                for kt in range(3):
                    nc.tensor.matmul(
                        aggm[:, mt * NG:(mt + 1) * NG],
                        wm[:, O_W2 + kt * 384 + mt * 128: O_W2 + kt * 384 + (mt + 1) * 128],
                        hsum_s[:, kt, :],
                        start=(kt == 0), stop=(kt == 2))
            # x1p = x + aggm/SCALE + b2*K/SCALE
            x1p = work1.tile([128, 3, NG], F32R, tag="x1p", name="x1p")
            for mt in range(3):
                nc.vector.tensor_scalar(
                    x1p[:, mt, :], aggm[:, mt * NG:(mt + 1) * NG],
                    1.0 / SCALE, b2s[:, mt:mt + 1], op0=OP.mult, op1=OP.add)
            nc.vector.tensor_add(x1p[:], x1p[:].bitcast(F32),
                                 xg[g][:].bitcast(F32))
            x1 = layernorm(x1p, lnw, 0, g, masked=False, tag=f"x1_{g}",
                           tp_tile=tp_tile)
            x1b = work1.tile([128, 3, NG], BF16, tag=f"x1b{g}", bufs=2,
                             name="x1b")
            nc.vector.tensor_copy(x1b[:], x1[:].bitcast(F32))

            # dense MLP: d0 = gelu(x1 @ dw0 + db0); d1 = d0 @ dw1 + db1
            d0g = work1.tile([128, 12, NG], BF16, tag="d0g", name="d0g")
            for r in range(4):
                dp = tp_tile("dp")
                for j in range(3):
                    mt = r * 3 + j
                    reg = dp[:, j * NG:(j + 1) * NG]
                    for kt in range(3):
                        nc.tensor.matmul(
                            reg,
                            wm[:, O_DW0 + kt * 1536 + mt * 128: O_DW0 + kt * 1536 + (mt + 1) * 128],
                            x1b[:, kt, :],
                            start=(kt == 0), stop=(kt == 2))
                    nc.scalar.activation(d0g[:, mt, :], reg, act,
                                         bias=wb[:, O_DB0 + mt:O_DB0 + mt + 1])
            d1p = tp_tile("d1p")
            for mt in range(3):
                for kt in range(12):
                    nc.tensor.matmul(
                        d1p[:, mt * NG:(mt + 1) * NG],
                        wm[:, O_DW1 + kt * 384 + mt * 128: O_DW1 + kt * 384 + (mt + 1) * 128],
                        d0g[:, kt, :],
                        start=(kt == 0), stop=(kt == 11))
            x2p = work1.tile([128, 3, NG], F32R, tag="x2p", name="x2p")
            for mt in range(3):
                nc.vector.tensor_scalar(
                    x2p[:, mt, :], d1p[:, mt * NG:(mt + 1) * NG],
                    1.0, wb[:, O_DB1 + mt:O_DB1 + mt + 1],
                    op0=OP.mult, op1=OP.add)
            nc.vector.tensor_add(x2p[:], x2p[:].bitcast(F32),
                                 x1[:].bitcast(F32))
            xo = layernorm(x2p, lnw, 1, g, masked=True, tag=f"x{g}",
                           tp_tile=tp_tile)
            xg[g] = xo
            if l + 1 < layers:
                xb = xpool.tile([128, 3, NG], BF16, tag=f"xb{g}", name=f"xb{g}")
                nc.vector.tensor_copy(xb[:], xo[:].bitcast(F32))
                xbg[g] = xb
                return prep_xw(l + 1, g, wm_next, wb_next)
            nc.sync.dma_start(out_p[:, :, g * NG:(g + 1) * NG],
                              xo[:].bitcast(F32))
            return None

        # ================= pipeline =================
        g_tiles = {0: compute_g(0), 1: compute_g(1), 2: compute_g(2)}
        wms = {0: load_weights(0), 1: load_weights(1)}
        b2s_all = {}

        def get_b2s(l, wb):
            if l not in b2s_all:
                t = small.tile([128, 3], F32, tag=f"b2s{l % 2}", name=f"b2s{l}")
                nc.vector.tensor_scalar_mul(t[:], wb[:, O_B2:O_B2 + 3],
                                            K / SCALE)
                b2s_all[l] = t
            return b2s_all[l]

        xw8s = {}
        wm0, _, wb0, _, _ = wms[0]
        for g in range(2):
            xw8s[g] = prep_xw(0, g, wm0, wb0)

        for l in range(layers):
            wm, wm8, wb, lnw, b1r = wms[l]
            b2s = get_b2s(l, wb)
            if l + 1 < layers:
                if l + 1 not in wms:
                    wms[l + 1] = load_weights(l + 1)
                wm_next, _, wb_next, _, _ = wms[l + 1]
            else:
                wm_next = wb_next = None
            for g in range(2):
                hsum = kloop(l, g, g_tiles[l], xw8s[g], wm8, wb, b1r)
                xw8s[g] = tail(l, g, hsum, wm, wb, lnw, wm_next, wb_next,
                               final=(l == layers - 1 and g == 1))

    nc.finalize()
    return nc


def _get_nc():
    if "nc" not in _NC_CACHE:
        _NC_CACHE["nc"] = _emit()
    return _NC_CACHE["nc"]


def _fm(w):
    """[in, out] fp32 -> [128, n_kt*out] (feature-major lhsT blob columns)."""
    i, o = w.shape
    return np.ascontiguousarray(
        w.reshape(i // 128, 128, o).transpose(1, 0, 2).reshape(128, -1))


def _wrap_idx(vals):
    """[n] int -> [128, n//16] int16 wrapped (i -> [i%16, i//16]) x8 replicas."""
    n = vals.shape[0]
    w = np.ascontiguousarray(vals.reshape(n // 16, 16).T).astype(np.int16)
    return np.tile(w, (8, 1))


def _marshal(inputs):
    nf = np.asarray(inputs["node_features"], np.float32)
    ef = np.asarray(inputs["edge_features"], np.float32)
    idx = np.asarray(inputs["neighbor_indices"])
    mask = np.asarray(inputs["mask"], np.float32)

    # replicated tensors
    nfm = np.ascontiguousarray(
        nf.astype(BF).reshape(N, 3, 128).transpose(2, 1, 0))           # [128,3,N]
    f8np = mybir.dt.np(mybir.dt.float8e4)
    wm = np.empty((L, 128, WMC), BF)
    wm8 = np.empty((L, 128, W8C), f8np)
    wb = np.empty((L, 128, 24), np.float32)
    wn = np.empty((128, L, 3, 384), BF)
    lnpk = np.empty((L, 1, 1920), np.float32)
    b1r_m = np.empty((L, 1, 384), np.float32)
    for l in range(L):
        w0 = np.asarray(inputs["msg_w0"], np.float32)[l]
        cols = [
            _fm(w0[0:384]),
            _fm(np.asarray(inputs["msg_w2"], np.float32)[l]),
            _fm(np.asarray(inputs["dense_w0"], np.float32)[l]),
            _fm(np.asarray(inputs["dense_w1"], np.float32)[l]),
        ]
        wm[l] = np.concatenate(cols, axis=1).astype(BF)
        w0e = _fm(w0[384:768])
        w1f = _fm(np.asarray(inputs["msg_w1"], np.float32)[l])
        c8 = []
        for W, sc in ((w0e, SC_W0E), (w1f, SC_W1)):
            q = (W * sc).astype(f8np)
            d = (W * sc - q.astype(np.float32)).astype(f8np)
            for mt in range(3):
                blk = lambda A, kt: A[:, kt * 384 + mt * 128:
                                      kt * 384 + (mt + 1) * 128]
                c8 += [blk(q, 0), blk(q, 1), blk(d, 1), blk(d, 2),
                       blk(d, 0), blk(q, 2)]
        wm8[l] = np.concatenate(
            [c.astype(f8np) for c in c8], axis=1)
        bcols = [
            np.asarray(inputs["msg_b0"], np.float32)[l].reshape(3, 128).T,
            np.asarray(inputs["msg_b1"], np.float32)[l].reshape(3, 128).T,
            np.asarray(inputs["msg_b2"], np.float32)[l].reshape(3, 128).T,
            np.asarray(inputs["dense_b0"], np.float32)[l].reshape(12, 128).T,
            np.asarray(inputs["dense_b1"], np.float32)[l].reshape(3, 128).T,
        ]
        wb[l] = np.concatenate(bcols, axis=1)
        wn[:, l] = w0[1152:1536].astype(BF).reshape(3, 128, 384).transpose(1, 0, 2)
        lnpk[l, 0] = np.concatenate([
            np.asarray(inputs["ln1_w"], np.float32)[l],
            np.asarray(inputs["ln2_w"], np.float32)[l],
            np.asarray(inputs["ln1_b"], np.float32)[l],
            np.asarray(inputs["ln2_b"], np.float32)[l],
            np.asarray(inputs["msg_b1"], np.float32)[l] * SC_W1])
        b1r_m[l, 0] = np.asarray(inputs["msg_b1"], np.float32)[l] * SC_W1
    consts = np.zeros((128, 769), np.float32)
    consts[:, 0] = 1.0
    consts[0, 1:769] = 1.0
    constsb = (np.eye(128, dtype=np.float32) * SC_W0E).astype(BF)
    consts8 = np.broadcast_to(np.eye(128, dtype=np.float32), (2, 128, 128))
    consts8 = np.ascontiguousarray(
        consts8.transpose(1, 0, 2)).astype(f8np)

    in_maps = []
    for c in range(NCORES):
        lo = slice(c * NLOC, (c + 1) * NLOC)
        efc = ef[lo]                                       # [256,48,384]
        idc = idx[lo]                                      # [256,48]
        edge = np.empty((2, NCH, 128, 3, CH), f8np)
        gidx = np.empty((128, 2, NCH, CH // 16), np.int16)
        for g in range(2):
            gs = slice(g * NG, (g + 1) * NG)
            E = efc[gs].transpose(1, 0, 2).reshape(TG, 384)    # k-major tokens
            edge[g] = (E.reshape(NCH, 8, NG, 3, 128)
                       .transpose(0, 4, 3, 1, 2).reshape(NCH, 128, 3, CH)
                       .astype(f8np))
            idx_k = np.ascontiguousarray(idc[gs].T).reshape(TG)
            for cc in range(NCH):
                gidx[:, g, cc, :] = _wrap_idx(idx_k[cc * CH:(cc + 1) * CH])
        x0 = np.ascontiguousarray(
            nf[lo].reshape(NLOC, 3, 128).transpose(2, 1, 0))   # [128,3,256]
        in_maps.append(dict(
            edge=edge, nfm=nfm, gidx=gidx, wm=wm, wb=wb, wn=wn, lnpk=lnpk,
            consts=consts, constsb=constsb, consts8=consts8, wm8=wm8,
            b1r=b1r_m, x0=x0,
            mask=np.ascontiguousarray(mask[lo])[None, :]))
    return in_maps


def _unshard(results):
    out = np.empty((N, NF), np.float32)
    for c in range(NCORES):
        xfm = results[c]["out_x"]                          # [128,3,256]
        out[c * NLOC:(c + 1) * NLOC] = xfm.transpose(2, 1, 0).reshape(NLOC, NF)
    return out


def kernel(**inputs):
    nc = _get_nc()
    in_maps = _marshal(inputs)
    res = run_bass_kernel_spmd(nc, in_maps, list(range(NCORES)), trace=False)
    return _unshard(res.results)
